# revision 1
# baseline (speedup 1.0000x reference)
"""Informer encoder (ProbSparse attention) on 8 NeuronCores.

Sharding: data-parallel over batch (B=8 -> 1 element per core); every core
holds the full weight set.

The ProbSparse sampling step is reformulated gather-free (Trainium indirect
loads at 4B granularity are catastrophically slow):
  - qk_sample max term:  max_j(q @ k^T + maskbias)  with maskbias[l,j] = 0 if
    j in index_sample[l], else -inf  (host-precomputed from index_sample).
  - qk_sample sum term:  rowwise_dot(q, Cmat @ k)   with Cmat[l,j] =
    multiplicity of j in index_sample[l]  (host-precomputed).
  - top-u gather/scatter: one-hot matmuls instead of take_along_axis/.at.set.
"""

import functools
import hashlib

import jax
import jax.numpy as jnp
import numpy as np

B, L, C_IN, D, H, DH, DFF, EL, NC = 8, 2048, 3, 256, 8, 32, 1024, 3, 10
FACTOR = 5
SAMPLE_K = min(FACTOR * int(np.ceil(np.log(L))), L)  # 40
U_TOP = min(FACTOR * int(np.ceil(np.log(L))), L)     # 40
SCALE = 1.0 / float(np.sqrt(DH))

_WEIGHT_KEYS = ('pe', 'conv_w', 'Wq', 'bq', 'Wk', 'bk', 'Wv', 'bv', 'Wo',
                'bo', 'c1w', 'c1b', 'c2w', 'c2b', 'n1g', 'n1b', 'n2g', 'n2b',
                'ng', 'nb', 'fcw', 'fcb')


def _layer_norm(x, g, b, eps=1e-5):
    m = jnp.mean(x, axis=-1, keepdims=True)
    v = jnp.mean((x - m) ** 2, axis=-1, keepdims=True)
    return (x - m) / jnp.sqrt(v + eps) * g + b


def _prob_attention(q, k, v, maskbias, cmat):
    # q,k,v: [H,L,DH] (single batch element)
    s_full = jnp.einsum('hld,hjd->hlj', q, k)               # [H,L,L]
    max_term = jnp.max(s_full + maskbias[None], axis=-1)    # [H,L]
    ksum = jnp.einsum('lj,hjd->hld', cmat, k)               # [H,L,DH]
    sum_term = jnp.einsum('hld,hld->hl', q, ksum)
    m = max_term - sum_term / L                             # [H,L]
    _, top_idx = jax.lax.top_k(m, U_TOP)                    # [H,u]
    sel = jax.nn.one_hot(top_idx, L, dtype=q.dtype)         # [H,u,L]
    q_red = jnp.einsum('hul,hld->hud', sel, q)              # [H,u,DH]
    scores = jnp.einsum('hud,hkd->huk', q_red, k) * SCALE
    attn = jax.nn.softmax(scores, axis=-1)
    upd = jnp.einsum('huk,hkd->hud', attn, v)               # [H,u,DH]
    mean_v = jnp.mean(v, axis=1, keepdims=True)             # [H,1,DH]
    ctx = mean_v + jnp.einsum('hul,hud->hld', sel, upd - mean_v)
    return jnp.transpose(ctx, (1, 0, 2)).reshape(L, D)


def _token_embed(x, conv_w):
    # x: [L, C_IN]; circular conv1d k=3 pad=1, no bias
    x_pad = jnp.concatenate([x[-1:], x, x[:1]], axis=0)     # [L+2, C_IN]
    y = jnp.zeros((L, D), x.dtype)
    for kk in range(3):
        y = y + jnp.einsum('lc,dc->ld', x_pad[kk:kk + L], conv_w[:, :, kk])
    return y


def _forward_one(x_enc, x_mark_enc, maskbias, cmat, pe, conv_w, Wq, bq, Wk,
                 bk, Wv, bv, Wo, bo, c1w, c1b, c2w, c2b, n1g, n1b, n2g, n2b,
                 ng, nb, fcw, fcb):
    x = _token_embed(x_enc, conv_w) + pe                    # [L, D]
    for i in range(EL):
        q = (x @ Wq[i].T + bq[i]).reshape(L, H, DH).transpose(1, 0, 2)
        k = (x @ Wk[i].T + bk[i]).reshape(L, H, DH).transpose(1, 0, 2)
        v = (x @ Wv[i].T + bv[i]).reshape(L, H, DH).transpose(1, 0, 2)
        att = _prob_attention(q, k, v, maskbias, cmat) @ Wo[i].T + bo[i]
        x = x + att
        x = _layer_norm(x, n1g[i], n1b[i])
        y = jax.nn.gelu(x @ c1w[i].T + c1b[i], approximate=False)
        y = y @ c2w[i].T + c2b[i]
        x = _layer_norm(x + y, n2g[i], n2b[i])
    x = _layer_norm(x, ng, nb)
    out = x * x_mark_enc[..., None]                         # [L, D]
    return jnp.einsum('ld,cld->c', out, fcw.reshape(NC, L, D)) + fcb


@functools.partial(jax.pmap, in_axes=(0, 0) + (None,) * (2 + len(_WEIGHT_KEYS)),
                   axis_name='b')
def _forward_pmap(x_enc, x_mark_enc, maskbias, cmat, *weights):
    return _forward_one(x_enc, x_mark_enc, maskbias, cmat, *weights)


_CACHE = {}


def _prep(inputs):
    idx = np.asarray(inputs['index_sample'])
    key_h = hashlib.sha1(idx.tobytes())
    for k in _WEIGHT_KEYS:
        key_h.update(np.ascontiguousarray(inputs[k]).tobytes())
    key = key_h.hexdigest()
    if key in _CACHE:
        return _CACHE[key]
    # maskbias: 0 where sampled, -inf elsewhere; cmat: sample multiplicity.
    cmat = np.zeros((L, L), np.float32)
    np.add.at(cmat, (np.repeat(np.arange(L), SAMPLE_K), idx.ravel()), 1.0)
    maskbias = np.where(cmat > 0, 0.0, -1e30).astype(np.float32)
    dev_args = [jax.device_put(jnp.asarray(maskbias)),
                jax.device_put(jnp.asarray(cmat))]
    dev_args += [jax.device_put(jnp.asarray(np.asarray(inputs[k], np.float32)))
                 for k in _WEIGHT_KEYS]
    _CACHE[key] = dev_args
    return dev_args


def kernel(**inputs):
    x_enc = np.asarray(inputs['x_enc'], np.float32)
    x_mark = np.asarray(inputs['x_mark_enc'], np.float32)
    dev_args = _prep(inputs)
    out = _forward_pmap(x_enc, x_mark, *dev_args)
    return np.asarray(jax.device_get(out), np.float32)

